# revision 1
# baseline (speedup 1.0000x reference)
"""CenterLoss (segment_reduce) Trainium2 Bass kernel.

loss = (1/N) * sum_{i,c: gt[i,c]>0} ||features[i] - centers[c]||^2
     = ( sum_i fsq[i]*rowcnt[i] + sum_c csq[c]*colcnt[c]
         - 2 * sum_{c,f} Z[c,f]*centers[c,f] ) / N
  with rowcnt = mask @ 1, colcnt = 1 @ mask, Z = mask^T @ features.

Per core (8-way data-parallel on rows, centers replicated):
  Z = mask^T @ [features_bf16 | 1 | fsq]  accumulated in PSUM over 64
  row-tiles of 128 (8 class chunks of 125 = 8 PSUM banks); columns
  256/257 of each chunk are colcnt / the fsq-weighted sums.  The
  int32->bf16 mask cast and the f32->bf16 feature cast both happen
  inside the SWDGE DMA; features are staged host-side as
  [features | 1.0 | fsq] so one DMA per group lands the matmul rhs
  directly in the resident [128, 64, 258] buffer and the per-tile
  device work is exactly 8 LDW+MM pairs — no ACT/DVE in the stream.
  The mask is fully SBUF-resident (128 KB/partition), so no buffer
  ever recycles and every instruction needs at most ONE sync wait
  (all this walrus build encodes).  Feature traffic is front-loaded
  so the last ~50 row tiles are a pure mask stream (the whole stream
  runs gapless at ~355 GB/s, the per-core HBM cap); the last two
  tiles use half-class DMAs so the PE tail after the final byte is
  ~4 matmuls.  Epilogue: 8 per-bank fused DVE scalar_tensor_tensor
  ops (Z*centers with accum_out, overlapping the final matmuls) + one
  strided PSUM copy + a single 12 KB HWDGE store.  The final scalar
  combine (csq in f64) runs on the host over the per-core partials,
  per the sharding hint's host all-reduce.
"""

import numpy as np

N_TOTAL = 65536
C = 1000
F = 256
NCORES = 8
NSH = N_TOTAL // NCORES  # 8192 rows per core
P = 128                  # partition tile (rows per matmul step)
T = NSH // P             # 64 row tiles per core
CCH = 125                # class chunk (PSUM partition dim)
NCH = C // CCH           # 8 class chunks == 8 PSUM banks
FS = F + 2               # rhs columns: features | ones | fsq
NSINGLE = 4              # trailing tiles with single/split mask DMAs
NPAIR = (T - NSINGLE) // 2

# feature tile groups (count) and the mask-op index each group's DMA is
# emitted after: everything is in flight by mask op 6 so the back half
# of the stream is mask-only.
FEAT_GROUPS = [2, 2, 4, 8, 12, 12, 12, 12]
FEAT_DUE = [0, 0, 1, 2, 3, 4, 5, 6]
CENT_DUE = 8


def build_bass():
    import concourse.bass as bass
    import concourse.mybir as mybir
    import concourse.tile as tile
    from contextlib import ExitStack

    f32 = mybir.dt.float32
    bf16 = mybir.dt.bfloat16
    i32 = mybir.dt.int32

    nc = bass.Bass(trn_type="TRN2")
    gt = nc.dram_tensor("gt", [NSH, C], i32, kind="ExternalInput")
    feat = nc.dram_tensor("features", [NSH, FS], f32, kind="ExternalInput")
    cent = nc.dram_tensor("centers", [C, F], f32, kind="ExternalInput")
    # single output partial [125, 24] = [t3 | colcnt | fsqsum] per
    # chunk: one store, so the kernel-tail drains keep a single wait.
    out = nc.dram_tensor("partial", [CCH, 3 * NCH], f32,
                         kind="ExternalOutput")

    gt_r = gt.rearrange("(t p) c -> t p c", p=P)
    gt_r2 = gt.rearrange("(j two p) c -> j p two c", two=2, p=P)
    feat_r = feat.rearrange("(t p) f -> p t f", p=P)
    # chunk k, partition p  <->  class k*CCH + p
    cent_r = cent.rearrange("(k p) f -> p k f", p=CCH)

    starts = []
    s = 0
    for g in FEAT_GROUPS:
        starts.append(s)
        s += g
    assert s == T

    feat_due = {}
    for gi, due in enumerate(FEAT_DUE):
        feat_due.setdefault(due, []).append(gi)

    with tile.TileContext(nc) as tc, ExitStack() as ctx:
        const = ctx.enter_context(tc.tile_pool(name="const", bufs=1))
        ep = ctx.enter_context(tc.tile_pool(name="ep", bufs=1))
        zp = ctx.enter_context(tc.tile_pool(name="zp", bufs=1, space="PSUM"))

        # fully resident tensors: nothing recycles, so no WAR/WAW waits.
        mask_full = const.tile([P, T, C], bf16, name="mask_full")
        featx_full = const.tile([P, T, FS], bf16, name="featx_full")
        cent_t = const.tile([CCH, NCH, F], f32, name="cent_t")
        cent_obs = const.tile([1, 1], f32, name="cent_obs")

        # one PSUM tensor spanning all 8 banks: chunk k accumulates in
        # z_big[:, k, 0:FS]; bank stride 512 f32 keeps each matmul output
        # inside a single bank.
        z_big = zp.tile([CCH, NCH, 512], mybir.dt.float32, name="z_big")

        def emit_feat(gi):
            st, g = starts[gi], FEAT_GROUPS[gi]
            nc.gpsimd.dma_start(out=featx_full[:, st:st + g, :],
                                in_=feat_r[:, st:st + g, :])

        def tile_compute(t):
            for k in range(NCH):
                nc.tensor.matmul(
                    z_big[:, k, 0:FS],
                    lhsT=mask_full[:, t, k * CCH:(k + 1) * CCH],
                    rhs=featx_full[:, t, :],
                    start=(t == 0),
                    stop=(t == T - 1),
                )

        mop = 0  # mask-op index for feat/cent dues

        def emit_dues():
            for gi in feat_due.get(mop, ()):
                emit_feat(gi)
            if mop == CENT_DUE:
                nc.gpsimd.dma_start(out=cent_t, in_=cent_r)
                # chained 1-element DVE read of centers: DVE observes the
                # cent DMA here, so the epilogue reduce needs only the PE
                # wait (walrus encodes a single wait per instruction).
                nc.vector.tensor_copy(out=cent_obs, in_=cent_t[0:1, 0, 0:1])

        for j in range(NPAIR):
            nc.gpsimd.dma_start(out=mask_full[:, 2 * j:2 * j + 2, :],
                                in_=gt_r2[j])
            emit_dues()
            mop += 1
            tile_compute(2 * j)
            tile_compute(2 * j + 1)
        for t in range(2 * NPAIR, T):
            if t >= T - 2:
                # half-class DMAs: chunks 0-3 matmul while classes
                # 500:1000 are still in flight, shrinking the PE tail
                # after the final HBM byte to ~4 matmuls.
                half = C // 2
                nc.gpsimd.dma_start(out=mask_full[:, t, 0:half],
                                    in_=gt_r[t][:, 0:half])
                nc.gpsimd.dma_start(out=mask_full[:, t, half:C],
                                    in_=gt_r[t][:, half:C])
            else:
                nc.gpsimd.dma_start(out=mask_full[:, t, :], in_=gt_r[t])
            emit_dues()
            mop += 1
            tile_compute(t)

        # ---- epilogue: fused mul+reduce of Z against centers, per bank
        # (2D APs; bank k's reduce starts as soon as its stop-matmul
        # retires, overlapping the last tile's remaining matmuls) ----
        w = ep.tile([CCH, NCH, F], bf16, name="w")
        outb = ep.tile([CCH, 3 * NCH], f32, name="outb")
        for k in range(NCH):
            nc.vector.scalar_tensor_tensor(
                out=w[:, k, :],
                in0=z_big[:, k, 0:F],
                scalar=1.0,
                in1=cent_t[:, k, :],
                op0=mybir.AluOpType.bypass,
                op1=mybir.AluOpType.mult,
                accum_out=outb[:, k:k + 1],
            )
        # cols 8:24 = [colcnt | fsqsum] per chunk, interleaved (one
        # strided copy of the ones and fsq columns of each bank)
        nc.vector.tensor_copy(out=outb[0:CCH, NCH:3 * NCH],
                              in_=z_big[:, :, F:FS])
        nc.sync.dma_start(out=out[:, :], in_=outb)

    _fix_sync_waits(nc)
    return nc


def _fix_sync_waits(nc):
    """This walrus build encodes only ONE sync wait per compute/DMA
    instruction.  With every SBUF buffer fully resident (no recycling)
    each compute/DMA instruction naturally has at most one wait; the only
    multi-wait instructions left are the kernel-tail drains, which only
    need the completion sems of the DMAs that write DRAM outputs (every
    input DMA's completion is implied by its consumers, which the
    per-engine drains already order after).
    """
    out_sems = set()
    for f in nc.m.functions:
        for b in f.blocks:
            for inst in b.instructions:
                if (type(inst).__name__ == "InstDMACopy"
                        and inst.outs
                        and str(inst.outs[0].memsetref).startswith("partial")):
                    for u in inst.sync_info.on_update:
                        out_sems.add(u.ant_name)
    assert out_sems, "no output DMA found"

    for f in nc.m.functions:
        for b in f.blocks:
            for inst in b.instructions:
                si = inst.sync_info
                if si is None or len(si.on_wait) <= 1:
                    continue
                waits = list(si.on_wait)
                tn = type(inst).__name__
                if tn == "InstDrain":
                    keep = [w for w in waits if w.ant_name in out_sems]
                    assert keep, (
                        f"drain {inst.name}: no output-DMA wait among "
                        f"{[w.ant_name for w in waits]}")
                    inst.sync_info = type(si)(on_wait=keep,
                                              on_update=si.on_update)
                else:
                    raise AssertionError(
                        f"unexpected multi-wait {tn} {inst.name} "
                        f"({inst.engine.name}): "
                        f"{[w.ant_name for w in waits]}")


def _shard_inputs(inputs):
    gt = np.ascontiguousarray(np.asarray(inputs["gt"], dtype=np.int32))
    features = np.asarray(inputs["features"], dtype=np.float32)
    centers = np.ascontiguousarray(np.asarray(inputs["centers"], dtype=np.float32))
    # stage [features | 1.0 | fsq]: the ones and fsq columns ride the
    # feature DMA and become the colcnt / fsq-sum columns of each PSUM
    # chunk (t1 = sum_c (mask^T fsq)[c], t2 needs colcnt).
    featx = np.empty((N_TOTAL, FS), dtype=np.float32)
    featx[:, 0:F] = features
    featx[:, F] = 1.0
    featx[:, F + 1] = (features.astype(np.float64) ** 2).sum(axis=1)
    in_maps = []
    for c in range(NCORES):
        sl = slice(c * NSH, (c + 1) * NSH)
        in_maps.append({
            "gt": gt[sl],
            "features": featx[sl],
            "centers": centers,
        })
    return in_maps


def _combine(results, centers):
    """Host-side scalar combine (the all-reduce of the sharding hint).

    Per-core output: partial [125, 24].  Col k = t3 partial for chunk k
    = sum_f Z[k*125+p, f]*centers[k*125+p, f]; cols 8:24 interleave
    colcnt[p,k] (8+2k) and fsqsum[p,k] (9+2k) per chunk.
    """
    csq = (centers.astype(np.float64) ** 2).sum(axis=1)  # [C]
    csq_pk = csq.reshape(NCH, CCH).T                     # [125, 8]
    t1 = t2 = t3 = 0.0
    for r in results:
        part = np.asarray(r["partial"], dtype=np.float64)
        t3 += part[:, 0:NCH].sum()
        t2 += (part[:, NCH:3 * NCH:2] * csq_pk).sum()
        t1 += part[:, NCH + 1:3 * NCH:2].sum()
    return (t1 + t2 - 2.0 * t3) / N_TOTAL


def run_spmd(inputs, trace=False):
    """Compile + run on all 8 cores. Returns (loss_scalar, BassKernelResults)."""
    from concourse.bass_utils import run_bass_kernel_spmd

    nc = build_bass()
    in_maps = _shard_inputs(inputs)
    res = run_bass_kernel_spmd(
        nc, in_maps, core_ids=list(range(NCORES)), trace=trace,
    )
    loss = _combine(res.results,
                    np.asarray(inputs["centers"], dtype=np.float32))
    return np.array(np.float32(loss), dtype=np.float32), res


def kernel(**inputs):
    loss, _ = run_spmd(inputs, trace=False)
    return loss


if __name__ == "__main__":
    # quick CoreSim numerical check on core 0's shard
    from concourse.bass_interp import CoreSim

    rng = np.random.default_rng(0)
    gt = (rng.integers(0, 2, size=(NSH, C))).astype(np.int32)
    features = rng.standard_normal((NSH, F)).astype(np.float32)
    centers = rng.standard_normal((C, F)).astype(np.float32)

    featx = np.empty((NSH, FS), dtype=np.float32)
    featx[:, 0:F] = features
    featx[:, F] = 1.0
    featx[:, F + 1] = (features.astype(np.float64) ** 2).sum(axis=1)

    nc = build_bass()
    sim = CoreSim(nc, require_finite=True, require_nnan=True)
    sim.tensor("gt")[:] = gt
    sim.tensor("features")[:] = featx
    sim.tensor("centers")[:] = centers
    sim.simulate()

    class _R:
        results = [{"partial": np.asarray(sim.tensor("partial"))}]

    got = _combine(_R.results, centers) * N_TOTAL

    mask = (gt > 0).astype(np.float64)
    f64, c64 = features.astype(np.float64), centers.astype(np.float64)
    dist = (
        (f64 * f64).sum(1)[:, None]
        + (c64 * c64).sum(1)[None, :]
        - 2.0 * (f64 @ c64.T)
    )
    want = float((mask * dist).sum())
    print(f"sim partial sum = {got:.6e}  want = {want:.6e}  rel = {abs(got - want) / abs(want):.3e}")



# revision 8
# speedup vs baseline: 2.2027x; 2.2027x over previous
"""CenterLoss (segment_reduce) Trainium2 Bass kernel — fp8 DoubleRow rewrite.

loss·N = t1 + t2 - 2·t3 with
  t1 = sum_i fsq[i]·rowcnt[i],  t2 = sum_c csq[c]·colcnt[c],
  t3 = sum_{c,f} Z[c,f]·centers[c,f],  Z = mask^T @ features.

Everything is folded into ONE device contraction Z2 = X^T @ mask with the
augmented X = [features | 1 | (fsq-256)/2] (258 cols, fp8) and an epilogue
elementwise reduce against W = [-2·centers^T ; csq+256 ; 2]:
  sum(Z2 ∘ W) = -2·t3 + (t2 + 256·T0) + (t1 - 256·T0) = N·loss_partial.
(The fsq column is mean-centred so its fp8 quantisation error is ~0.8% of
fsq instead of ~6%; the 256·T0 cross-terms cancel via the csq+256 row.)

Per core (8-way data-parallel on rows):
  - mask staged host-side as fp8 bytes (0.0/1.0 exact): 1 B/elt instead of
    the baseline's 4 B/elt int32 — 4x less HBM traffic on the dominant
    stream (8.26 MB/core vs 32.8 MB).
  - X is the STATIONARY matmul operand (3 chunks of 128|128|2 cols,
    weights reused across class halves so LDWEIGHTS hides), the mask
    STREAMS through the PE.  perf_mode=DoubleRow contracts 256 rows per
    pass (2 fp8 MACs/cell/cycle) — 32 double-tiles of 6 matmuls each,
    accumulating in 6 PSUM banks (class halves 512|488 to stay within the
    2 KB bank limit).
  - Epilogue: 6 DVE scalar_tensor_tensor mult+accum ops against the
    staged W -> one [128, 6] partial per core; host sums in f64 and
    divides by N (the all-reduce of the sharding hint).
All tensors are staged host-side in the exact [partition, tile, col] SBUF
layout, so every DMA is a contiguous per-partition HWDGE copy (no casts,
no rearrange descriptors).
"""

import numpy as np

N_TOTAL = 65536
C = 1000
F = 256
NCORES = 8
NSH = N_TOTAL // NCORES  # 8192 rows per core
P = 128                  # partitions (rows per k-tile)
T = NSH // P             # 64 row tiles per core
TD = T // 2              # 32 DoubleRow tiles (256 rows each)
CP = 1008                # class dim padded to %16 for DR access patterns
FP = 272                 # featx col dim padded to %16 (258 used)
FS = F + 2               # used featx cols: features | ones | fsq-resid
H0, H1 = 512, C - 512    # class halves (PSUM bank = 512 f32)


def build_bass():
    import concourse.bass as bass
    import concourse.mybir as mybir
    import concourse.tile as tile
    from contextlib import ExitStack

    f32 = mybir.dt.float32
    bf16 = mybir.dt.bfloat16
    f8 = mybir.dt.float8e4
    DR = mybir.MatmulPerfMode.DoubleRow
    bypass = mybir.AluOpType.bypass
    mult = mybir.AluOpType.mult

    nc = bass.Bass(trn_type="TRN2")
    mask_d = nc.dram_tensor("mask", [P, T, CP], f8, kind="ExternalInput")
    featx_d = nc.dram_tensor("featx", [P, T, FP], f8, kind="ExternalInput")
    centw_d = nc.dram_tensor("centw", [P, 2, C], bf16, kind="ExternalInput")
    cento_d = nc.dram_tensor("cento", [2, C], f32, kind="ExternalInput")
    out_d = nc.dram_tensor("partial", [P, 6], f32, kind="ExternalOutput")

    with tile.TileContext(nc) as tc, ExitStack() as ctx:
        const = ctx.enter_context(tc.tile_pool(name="const", bufs=1))
        zp = ctx.enter_context(tc.tile_pool(name="zp", bufs=1, space="PSUM"))

        # fully resident SBUF tensors (nothing recycles)
        mask_full = const.tile([P, T, CP], f8, name="mask_full")
        featx_full = const.tile([P, T, FP], f8, name="featx_full")
        centw = const.tile([P, 2, C], bf16, name="centw")
        cento = const.tile([2, C], f32, name="cento")
        acc = const.tile([P, 6], f32, name="acc")
        junk = const.tile([P, H0], bf16, name="junk")
        obs = const.tile([1, 2], f32, name="obs")

        # 6 PSUM banks: 4 feature-chunk accумulators + 2 ones/fsq rows
        zf = zp.tile([P, 4, 512], f32, name="zf")
        zo = zp.tile([P, 2, 512], f32, name="zo")

        nc.vector.memset(acc, 0.0)

        def mask_dma(j):
            nc.sync.dma_start(out=mask_full[:, 2 * j:2 * j + 2, :],
                              in_=mask_d[:, 2 * j:2 * j + 2, :])

        def featx_dma(a, b):
            nc.sync.dma_start(out=featx_full[:, a:b, :],
                              in_=featx_d[:, a:b, :])

        # One FIFO HWDGE queue: featx front-loaded between the first mask
        # tiles, epilogue weights after mask 8; mask j's completion implies
        # everything emitted before it has landed.
        featx_dma(0, 8)
        mask_dma(0)
        mask_dma(1)
        featx_dma(8, 24)
        mask_dma(2)
        mask_dma(3)
        featx_dma(24, 44)
        mask_dma(4)
        mask_dma(5)
        featx_dma(44, 64)
        mask_dma(6)
        mask_dma(7)
        nc.sync.dma_start(out=centw, in_=centw_d[:, :, :])
        nc.sync.dma_start(out=cento, in_=cento_d[:, :])
        # chained 1-element DVE reads: DVE observes the epilogue-weight
        # DMAs here, so each epilogue STT later needs only its PE wait
        # (walrus encodes a limited number of sync waits per STT).
        nc.vector.tensor_copy(out=obs[0:1, 0:1], in_=centw[0:1, 0, 0:1])
        nc.vector.tensor_copy(out=obs[0:1, 1:2], in_=cento[0:1, 0:1])
        for j in range(8, TD):
            mask_dma(j)

        for j in range(TD):
            st = j == 0
            sp = j == TD - 1
            lA = featx_full[:, 2 * j:2 * j + 2, 0:128]
            lB = featx_full[:, 2 * j:2 * j + 2, 128:256]
            lO = featx_full[:, 2 * j:2 * j + 2, 256:258]
            r0 = mask_full[:, 2 * j:2 * j + 2, 0:H0]
            r1 = mask_full[:, 2 * j:2 * j + 2, H0:C]
            nc.tensor.matmul(zf[:, 0, 0:H0], lhsT=lA, rhs=r0,
                             start=st, stop=sp, perf_mode=DR)
            nc.tensor.matmul(zf[:, 1, 0:H1], lhsT=lA, rhs=r1,
                             start=st, stop=sp, perf_mode=DR)
            nc.tensor.matmul(zf[:, 2, 0:H0], lhsT=lB, rhs=r0,
                             start=st, stop=sp, perf_mode=DR)
            nc.tensor.matmul(zf[:, 3, 0:H1], lhsT=lB, rhs=r1,
                             start=st, stop=sp, perf_mode=DR)
            nc.tensor.matmul(zo[0:2, 0, 0:H0], lhsT=lO, rhs=r0,
                             start=st, stop=sp, perf_mode=DR)
            nc.tensor.matmul(zo[0:2, 1, 0:H1], lhsT=lO, rhs=r1,
                             start=st, stop=sp, perf_mode=DR)

        # ---- epilogue: fused mul+reduce of Z2 against the staged W ----
        def stt(i0, i1, slot, n, parts=P):
            nc.vector.scalar_tensor_tensor(
                out=junk[0:parts, 0:n],
                in0=i0,
                scalar=1.0,
                in1=i1,
                op0=bypass,
                op1=mult,
                accum_out=acc[0:parts, slot:slot + 1],
            )

        stt(zf[:, 0, 0:H0], centw[:, 0, 0:H0], 0, H0)
        stt(zf[:, 1, 0:H1], centw[:, 0, H0:C], 1, H1)
        stt(zf[:, 2, 0:H0], centw[:, 1, 0:H0], 2, H0)
        stt(zf[:, 3, 0:H1], centw[:, 1, H0:C], 3, H1)
        stt(zo[0:2, 0, 0:H0], cento[0:2, 0:H0], 4, H0, parts=2)
        stt(zo[0:2, 1, 0:H1], cento[0:2, H0:C], 5, H1, parts=2)

        nc.sync.dma_start(out=out_d[:, :], in_=acc)

    _fix_sync_waits(nc)
    return nc


def _fix_sync_waits(nc):
    """Strip provably-redundant same-engine semaphore self-waits.

    Tile encodes some cross-instruction deps as waits on the instruction's
    own engine semaphore at a value already reached by an EARLIER
    instruction on the same (in-order) engine — trivially satisfied by
    program order.  Walrus can only encode one sync wait on an STT, so
    these must go.  Every remaining compute instruction must have <=1
    wait (drains may keep several; walrus accepts that).
    """
    insts = []
    for f in nc.m.functions:
        for b in f.blocks:
            insts.extend(b.instructions)

    # which engines increment each semaphore
    updaters = {}
    out_sems = set()
    for inst in insts:
        si = inst.sync_info
        if si is None:
            continue
        for u in si.on_update:
            updaters.setdefault(u.ant_name, set()).add(inst.engine)
        if (type(inst).__name__ == "InstDMACopy" and inst.outs
                and str(inst.outs[0].memsetref).startswith("partial")):
            for u in si.on_update:
                out_sems.add(u.ant_name)
    assert out_sems, "no output DMA found"

    # cumulative per-engine increments in program order
    cum = {}
    for inst in insts:
        si = inst.sync_info
        tn = type(inst).__name__
        if si is None:
            continue
        if tn == "InstDrain" and len(si.on_wait) > 1:
            # kernel-tail drains only need the DRAM-output DMA's sem:
            # every input DMA's completion is implied by its consumers,
            # which the per-engine drains already order after.
            keep = [w for w in si.on_wait if w.ant_name in out_sems]
            assert keep, (
                f"drain {inst.name}: no output-DMA wait among "
                f"{[w.ant_name for w in si.on_wait]}")
            inst.sync_info = type(si)(on_wait=keep, on_update=si.on_update)
            si = inst.sync_info
        elif tn != "InstDrain" and len(si.on_wait) > 1:
            keep = []
            for w in si.on_wait:
                eng_cnt = cum.get((inst.engine, w.ant_name), 0)
                same_engine_only = updaters.get(w.ant_name) == {inst.engine}
                if (same_engine_only and w.wait_value is not None
                        and eng_cnt >= w.wait_value):
                    continue  # satisfied by in-order execution
                keep.append(w)
            assert len(keep) <= 1, (
                f"{tn} {inst.name} ({inst.engine}): still multi-wait "
                f"{[(w.ant_name, w.wait_value) for w in keep]}")
            inst.sync_info = type(si)(on_wait=keep, on_update=si.on_update)
            si = inst.sync_info
        for u in si.on_update:
            key = (inst.engine, u.ant_name)
            cum[key] = cum.get(key, 0) + (u.update_value or 1)


def _shard_inputs(inputs):
    import ml_dtypes

    fp8 = ml_dtypes.float8_e4m3
    gt = np.asarray(inputs["gt"])
    features = np.asarray(inputs["features"], dtype=np.float32)
    centers = np.asarray(inputs["centers"], dtype=np.float32)

    # mask: fp8 bytes, exactly 0.0 / 1.0 (0x00 / 0x38), laid out
    # [core, p, t, c] with the class dim zero-padded to CP.
    m8 = (gt != 0).astype(np.uint8) * np.uint8(0x38)
    mask_st = np.zeros((NCORES, P, T, CP), dtype=np.uint8)
    mask_st[..., :C] = m8.reshape(NCORES, T, P, C).transpose(0, 2, 1, 3)
    mask_st = mask_st.view(fp8)

    # featx: [features | 1 | (fsq-256)/2] in fp8, same layout, padded to FP
    fsq = (features.astype(np.float64) ** 2).sum(axis=1)
    fx = np.empty((N_TOTAL, FS), dtype=np.float32)
    fx[:, 0:F] = features
    fx[:, F] = 1.0
    fx[:, F + 1] = (fsq - 256.0) * 0.5
    fx8 = fx.astype(fp8)
    featx_st = np.zeros((NCORES, P, T, FP), dtype=fp8)
    featx_st[..., :FS] = fx8.reshape(NCORES, T, P, FS).transpose(0, 2, 1, 3)

    # epilogue weights (replicated): centw[p, k, c] = -2*centers[c, 128k+p]
    centw = np.ascontiguousarray(
        (-2.0 * centers.T).reshape(2, P, C).transpose(1, 0, 2)
    ).astype(ml_dtypes.bfloat16)
    csq = (centers.astype(np.float64) ** 2).sum(axis=1)
    cento = np.empty((2, C), dtype=np.float32)
    cento[0] = csq + 256.0
    cento[1] = 2.0

    in_maps = []
    for c in range(NCORES):
        in_maps.append({
            "mask": np.ascontiguousarray(mask_st[c]),
            "featx": np.ascontiguousarray(featx_st[c]),
            "centw": centw,
            "cento": cento,
        })
    return in_maps


def _combine(results):
    """Host-side scalar combine (the all-reduce of the sharding hint).

    Per-core partial [128, 6]: cols 0-3 are per-partition sums of
    Z2_feat ∘ (-2 centers^T) (= -2·t3), cols 4-5 are valid on partitions
    0-1 only: colcnt·(csq+256) and 2·fsq-resid sums (= t1 + t2).
    """
    total = 0.0
    for r in results:
        part = np.asarray(r["partial"], dtype=np.float64)
        total += part[:, 0:4].sum() + part[0:2, 4:6].sum()
    return total / N_TOTAL


def run_spmd(inputs, trace=False):
    """Compile + run on all 8 cores. Returns (loss_scalar, BassKernelResults)."""
    from concourse.bass_utils import run_bass_kernel_spmd

    nc = build_bass()
    in_maps = _shard_inputs(inputs)
    res = run_bass_kernel_spmd(
        nc, in_maps, core_ids=list(range(NCORES)), trace=trace,
    )
    loss = _combine(res.results)
    return np.array(np.float32(loss), dtype=np.float32), res


def kernel(**inputs):
    loss, _ = run_spmd(inputs, trace=False)
    return loss


if __name__ == "__main__":
    # quick CoreSim numerical check on core 0's shard
    from concourse.bass_interp import CoreSim

    rng = np.random.default_rng(0)
    gt = (rng.integers(0, 2, size=(N_TOTAL, C))).astype(np.int32)
    features = rng.standard_normal((N_TOTAL, F)).astype(np.float32)
    centers = rng.standard_normal((C, F)).astype(np.float32)

    in_maps = _shard_inputs({"gt": gt, "features": features,
                             "centers": centers})

    nc = build_bass()
    sim = CoreSim(nc, require_finite=True, require_nnan=True)
    for k, v in in_maps[0].items():
        sim.tensor(k)[:] = v
    sim.simulate()

    got = _combine([{"partial": np.asarray(sim.tensor("partial"))}]) * N_TOTAL

    sl = slice(0, NSH)
    mask = (gt[sl] > 0).astype(np.float64)
    f64 = features[sl].astype(np.float64)
    c64 = centers.astype(np.float64)
    dist = (
        (f64 * f64).sum(1)[:, None]
        + (c64 * c64).sum(1)[None, :]
        - 2.0 * (f64 @ c64.T)
    )
    want = float((mask * dist).sum())
    print(f"sim partial sum = {got:.6e}  want = {want:.6e}  "
          f"rel = {abs(got - want) / abs(want):.3e}")
